# revision 1
# baseline (speedup 1.0000x reference)
"""Trainium2 Bass kernel for nn_Compute_all_u (embedding gather + batched affine dot).

Computes, for each voxel v:
    u[v, :] = coeffs[e_v, 0, :] + x_v*coeffs[e_v, 1, :] + y_v*coeffs[e_v, 2, :] + z_v*coeffs[e_v, 3, :]
where e_v = voxels_elements[v], (x,y,z) = all_voxels_centroids[v].

Sharding: data-parallel over the voxel axis across 8 NeuronCores; the
24MB coeff table stays in HBM on every core.

Gather mechanism: the TRN2 runtime's indirect DMA honors exactly ONE
dynamic row-offset per SBUF partition per instruction (one descriptor
per partition; extra offsets in the offset AP are ignored — verified on
hardware). So each gather instruction fetches 128 rows of 12 floats:
partition p <- table[idx[p, k]] for instruction k. K instructions fill a
wide [128, 12K] tile, then 6 strided DVE tensor_tensor ops compute u for
all 128*K voxels of the tile at once.

Per-core voxel layout (host-side reshape, no permutation): voxel
v = t*128*K + p*K + k <-> tile t, partition p, slot k.
"""

import numpy as np

from concourse import bacc, bass, tile, mybir
from concourse.bass_utils import run_bass_kernel_spmd

N_VOXELS = 8_000_000
N_ELEM = 500_000
N_CORES = 8
P = 128

NPC = N_VOXELS // N_CORES  # 1_000_000 voxels per core
K = 489                    # voxels per partition per tile (gathers per tile)
TILES = 16                 # tiles per core
NPC_PAD = TILES * P * K    # 1_001_472


def build_nc(n_elem: int, k: int, tiles: int, bufs: int = 3) -> bass.Bass:
    # Bacc (not raw Bass): its compile pass splits multi-sem waits into
    # event semaphores — the TRN2 ISA allows at most one wait per
    # instruction and walrus codegen rejects Tile's raw output otherwise.
    nc = bacc.Bacc("TRN2")
    f32 = mybir.dt.float32

    idx_in = nc.declare_dram_parameter("idx", [tiles, P, k], mybir.dt.int32, isOutput=False)
    cent_in = nc.declare_dram_parameter("cent", [tiles, P, 3 * k], f32, isOutput=False)
    table = nc.declare_dram_parameter("table", [n_elem, 12], f32, isOutput=False)
    out = nc.declare_dram_parameter("out", [tiles, P, 3 * k], f32, isOutput=True)

    with tile.TileContext(nc) as tc:
        with (
            tc.tile_pool(name="io", bufs=bufs) as io_pool,
            tc.tile_pool(name="tmp", bufs=2) as tmp_pool,
        ):
            for t in range(tiles):
                idx_t = io_pool.tile([P, k], mybir.dt.int32, tag="idx")
                nc.sync.dma_start(out=idx_t[:], in_=idx_in[t])

                cent_t = io_pool.tile([P, 3 * k], f32, tag="cent")
                nc.sync.dma_start(out=cent_t[:], in_=cent_in[t])

                g = io_pool.tile([P, 12 * k], f32, tag="g")
                # one indirect DMA per 128 rows: partition p <- table[idx_t[p, kk]]
                for kk in range(k):
                    nc.gpsimd.indirect_dma_start(
                        out=g[:, 12 * kk:12 * (kk + 1)],
                        out_offset=None,
                        in_=table[:],
                        in_offset=bass.IndirectOffsetOnAxis(ap=idx_t[:, kk:kk + 1], axis=0),
                    )

                # g layout per voxel slot kk: [d=0..3][j=0..2]; centroids [j=0..2]
                gr = g[:].rearrange("p (k d j) -> p k d j", d=4, j=3)
                cr = cent_t[:].rearrange("p (k j) -> p k j", j=3)

                u = io_pool.tile([P, 3 * k], f32, tag="u")
                ur = u[:].rearrange("p (k j) -> p k j", j=3)

                mul = mybir.AluOpType.mult
                add = mybir.AluOpType.add

                tmp = tmp_pool.tile([P, 3 * k], f32, tag="t")
                tr = tmp[:].rearrange("p (k j) -> p k j", j=3)

                x_b = cr[:, :, 0:1].to_broadcast([P, k, 3])
                y_b = cr[:, :, 1:2].to_broadcast([P, k, 3])
                z_b = cr[:, :, 2:3].to_broadcast([P, k, 3])

                nc.vector.tensor_tensor(out=tr, in0=x_b, in1=gr[:, :, 1, :], op=mul)
                nc.vector.tensor_tensor(out=ur, in0=gr[:, :, 0, :], in1=tr, op=add)
                nc.vector.tensor_tensor(out=tr, in0=y_b, in1=gr[:, :, 2, :], op=mul)
                nc.vector.tensor_tensor(out=ur, in0=ur, in1=tr, op=add)
                nc.vector.tensor_tensor(out=tr, in0=z_b, in1=gr[:, :, 3, :], op=mul)
                nc.vector.tensor_tensor(out=ur, in0=ur, in1=tr, op=add)

                nc.sync.dma_start(out=out[t], in_=u[:])
    nc.finalize()
    return nc


_NC_CACHE: dict = {}


def _get_nc():
    key = (N_ELEM, K, TILES)
    if key not in _NC_CACHE:
        _NC_CACHE[key] = build_nc(*key)
    return _NC_CACHE[key]


def _shard_inputs(all_coeffs, all_voxels_centroids, voxels_elements):
    table = np.ascontiguousarray(all_coeffs.reshape(N_ELEM, 12), dtype=np.float32)
    in_maps = []
    for c in range(N_CORES):
        lo, hi = c * NPC, (c + 1) * NPC
        idx = np.zeros(NPC_PAD, dtype=np.int32)
        idx[:NPC] = voxels_elements[lo:hi].astype(np.int32)
        cent = np.zeros((NPC_PAD, 3), dtype=np.float32)
        cent[:NPC] = all_voxels_centroids[lo:hi]
        in_maps.append(
            {
                "idx": idx.reshape(TILES, P, K),
                "cent": cent.reshape(TILES, P, 3 * K),
                "table": table,
            }
        )
    return in_maps


def kernel(all_coeffs, all_voxels_centroids, voxels_elements, _trace=False, **run_kwargs):
    nc = _get_nc()
    in_maps = _shard_inputs(all_coeffs, all_voxels_centroids, voxels_elements)
    res = run_bass_kernel_spmd(
        nc, in_maps, core_ids=list(range(N_CORES)), trace=_trace, **run_kwargs
    )
    outs = []
    for c in range(N_CORES):
        o = res.results[c]["out"].reshape(NPC_PAD, 3)[:NPC]
        outs.append(o)
    full = np.concatenate(outs, axis=0).astype(np.float32)
    if _trace:
        return full, res
    return full

